# revision 14
# baseline (speedup 1.0000x reference)
"""ConceptBank (vq_codebook) Trainium2 kernel.

reference:
    qn = l2norm(query); bankn = l2norm(emb)
    sim = qn @ bankn.T          # (B, N) cosine sims
    p = softmax(sim, -1)        # (B, N)
    vec = p @ emb               # (B, D)
    returns (vec, p)

Strategy (8 cores, data-parallel over B):
  - host pre-transposes + casts query shard to qT [2, 128, B_loc] bf16
    (layout prep only; all math on device)
  - sim tile [128b, 512n] = qT.T @ bankT  (bf16 matmul, fp32 PSUM)
  - softmax without max-subtraction (cosines are in [-1, 1]):
    ACT exp (fused per-partition 1/||q|| scale) -> e in fp32r
  - 1/||q|| via Newton rsqrt on DVE (||q||^2 ~ chi2_256 => y0=1/16
    converges quadratically); ACT never switches off the Exp/Copy LUTs
  - PE-transpose e (fp32r) -> eT; vec_u = eT.T @ [emb | 1] (fp32r);
    the ones column lands the softmax denominator s in the vec PSUM
  - p = e * (1/s) (DVE), vec = vec_u * (1/s) (DVE/ACT alternating)
"""

import numpy as np
import ml_dtypes
from contextlib import ExitStack

import concourse.bass as bass
from concourse import bacc
import concourse.mybir as mybir
import concourse.tile as tile
from concourse.bass_utils import run_bass_kernel_spmd
from concourse.masks import make_identity

F32 = mybir.dt.float32
F32R = mybir.dt.float32r
BF16 = mybir.dt.bfloat16

B, N, D = 262144, 512, 256
NCORES = 8
B_LOC = B // NCORES          # 32768 rows per core
BLK = 2048                   # rows per macro-block
NSUB = BLK // 128            # 16 subtiles of 128 rows
DE = D + 2                   # emb free dim padded with ones columns


def _build(b_loc=B_LOC):
    nblk = b_loc // BLK
    nc = bacc.Bacc(None, target_bir_lowering=False)
    qt = nc.declare_dram_parameter("qt", [2, 128, b_loc], BF16, isOutput=False)
    emb = nc.declare_dram_parameter("emb", [N, D], F32, isOutput=False)
    p_out = nc.declare_dram_parameter("p", [b_loc, N], F32, isOutput=True)
    v_out = nc.declare_dram_parameter("vec", [b_loc, D], F32, isOutput=True)

    with ExitStack() as ctx:
        tc = ctx.enter_context(tile.TileContext(nc))
        const = ctx.enter_context(tc.tile_pool(name="const", bufs=1))
        qpool = ctx.enter_context(tc.tile_pool(name="qpool", bufs=2))
        opool = ctx.enter_context(tc.tile_pool(name="opool", bufs=2))
        epool = ctx.enter_context(tc.tile_pool(name="epool", bufs=6))
        tpool = ctx.enter_context(tc.tile_pool(name="tpool", bufs=3))
        spool = ctx.enter_context(tc.tile_pool(name="spool", bufs=2))
        ps_sim = ctx.enter_context(tc.tile_pool(name="ps_sim", bufs=3, space="PSUM"))
        ps_pt = ctx.enter_context(tc.tile_pool(name="ps_pt", bufs=2, space="PSUM"))
        ps_vec = ctx.enter_context(tc.tile_pool(name="ps_vec", bufs=2, space="PSUM"))
        ps_ss = ctx.enter_context(tc.tile_pool(name="ps_ss", bufs=1, space="PSUM"))

        # ---------------- prep: bank normalize + transpose ----------------
        emb_sb = const.tile([128, 4, D], F32)          # emb rows n=128j+p
        nc.sync.dma_start(out=emb_sb,
                          in_=emb.rearrange("(j p) d -> p j d", p=128))
        # emb_ext: [emb | 1 1] rounded to fp32r (vec-matmul rhs; the ones
        # columns accumulate the softmax denominator)
        emb_ext = const.tile([128, 4, DE], F32R)
        ones_f = const.tile([128, 2], F32)
        nc.vector.memset(ones_f, 1.0)
        for j in range(4):
            nc.scalar.copy(emb_ext[:, j, 0:D], emb_sb[:, j, :])
            nc.vector.tensor_copy(emb_ext[:, j, D:DE], ones_f)

        scr = const.tile([128, D], F32)
        ssb = const.tile([128, 4], F32)
        for j in range(4):
            # sum of squares along free dim via ACT Square + fused accum
            nc.scalar.activation(scr, emb_sb[:, j, :],
                                 mybir.ActivationFunctionType.Square,
                                 accum_out=ssb[:, j:j + 1])
        nbn = const.tile([128, 4], F32)
        nc.scalar.sqrt(nbn, ssb)
        rbn = const.tile([128, 4], F32)
        nc.vector.reciprocal(rbn, nbn)

        bankn = const.tile([128, 4, D], BF16)          # normalized bank, bf16
        for j in range(4):
            nc.vector.tensor_scalar_mul(bankn[:, j, :], emb_sb[:, j, :],
                                        rbn[:, j:j + 1])

        ident_h = const.tile([128, 128], BF16)
        make_identity(nc, ident_h)
        ident_f = const.tile([128, 128], F32)
        make_identity(nc, ident_f)
        ident_r = const.tile([128, 128], F32R)
        nc.vector.tensor_copy(ident_r, ident_f)
        ones_h = const.tile([128, 1], BF16)
        nc.vector.memset(ones_h, 1.0)

        # bankT [128d, k, 512n] bf16: transpose of bankn
        bankT = const.tile([128, 2, N], BF16)
        for j in range(4):
            for k in range(2):
                tp = ps_pt.tile([128, 128], BF16, tag="pt")
                nc.tensor.transpose(tp, bankn[:, j, 128 * k:128 * (k + 1)],
                                    ident_h)
                nc.vector.tensor_copy(bankT[:, k, 128 * j:128 * (j + 1)], tp)

        # ---------------- main loop ----------------
        for blk in range(nblk):
            b0 = blk * BLK
            qt_sb = qpool.tile([128, 2, BLK], BF16)
            nc.sync.dma_start(out=qt_sb, in_=qt[:, :, b0:b0 + BLK]
                              .rearrange("k p b -> p k b"))
            # squares for ||q||^2 (bf16: fine, no cancellation in the sum)
            qsq = qpool.tile([128, 2, BLK], BF16)
            nc.vector.tensor_mul(qsq, qt_sb, qt_sb)

            # ss[:, i] = sum_d qT[d, b]^2 for subtile i (PE ones-matmul)
            ss_ps = ps_ss.tile([128, NSUB], F32)
            for i in range(NSUB):
                for k in range(2):
                    nc.tensor.matmul(ss_ps[:, i:i + 1],
                                     qsq[:, k, 128 * i:128 * (i + 1)],
                                     ones_h, start=(k == 0), stop=(k == 1))
            # rq = 1/sqrt(ss) via Newton on DVE (ACT stays on Exp/Copy LUTs)
            ss_sb = spool.tile([128, NSUB], F32)
            nc.vector.tensor_copy(ss_sb, ss_ps)
            rq = spool.tile([128, NSUB], F32)
            nc.vector.memset(rq, 0.0625)               # 1/sqrt(256)
            t0 = spool.tile([128, NSUB], F32)
            for _ in range(3):
                nc.vector.tensor_mul(t0, rq, rq)       # y^2
                nc.vector.tensor_mul(t0, t0, ss_sb)    # a*y^2
                nc.vector.tensor_scalar(t0, t0, -0.5, 1.5,
                                        op0=mybir.AluOpType.mult,
                                        op1=mybir.AluOpType.add)
                nc.vector.tensor_mul(rq, rq, t0)       # y *= 1.5 - a*y^2/2

            p_sb = opool.tile([128, NSUB, N], F32)
            v_sb = opool.tile([128, NSUB, D], F32)
            rs_sb = spool.tile([128, NSUB], F32)

            for i in range(NSUB):
                # sim = qT.T @ bankT  (accumulate over 2 d-chunks)
                sim_ps = ps_sim.tile([128, N], F32)
                for k in range(2):
                    nc.tensor.matmul(sim_ps,
                                     qt_sb[:, k, 128 * i:128 * (i + 1)],
                                     bankT[:, k, :],
                                     start=(k == 0), stop=(k == 1))
                # e = exp(sim/||q||), rounded to fp32r
                e_sb = epool.tile([128, N], F32R)
                nc.scalar.activation(e_sb, sim_ps,
                                     mybir.ActivationFunctionType.Exp,
                                     scale=rq[:, i:i + 1])
                # eT via PE transpose (fp32r)
                pt_ps = ps_pt.tile([128, N], F32R, tag="pt")
                for j in range(4):
                    nc.tensor.transpose(pt_ps[:, 128 * j:128 * (j + 1)],
                                        e_sb[:, 128 * j:128 * (j + 1)],
                                        ident_r)
                pt_sb = tpool.tile([128, N], F32R)
                if i % 2 == 0:
                    nc.scalar.copy(pt_sb, pt_ps)
                else:
                    nc.vector.tensor_copy(pt_sb, pt_ps)
                # vec_u = eT.T @ [emb|1] (fp32r, accumulate 4 n-chunks);
                # column D collects s = sum_n e
                vec_ps = ps_vec.tile([128, 512], F32)
                for j in range(4):
                    nc.tensor.matmul(vec_ps[:, 0:DE],
                                     pt_sb[:, 128 * j:128 * (j + 1)],
                                     emb_ext[:, j, :],
                                     start=(j == 0), stop=(j == 3))
                # rs = 1/s
                nc.vector.reciprocal(rs_sb[:, i:i + 1],
                                     vec_ps[:, D:D + 1])
                # p = e * (1/s)
                nc.vector.tensor_scalar_mul(p_sb[:, i, :], e_sb,
                                            rs_sb[:, i:i + 1])
                # vec = vec_u * (1/s)  (alternate ACT-Copy / DVE)
                if i % 2 == 0:
                    nc.vector.tensor_scalar_mul(v_sb[:, i, :],
                                                vec_ps[:, 0:D],
                                                rs_sb[:, i:i + 1])
                else:
                    nc.scalar.activation(v_sb[:, i, :], vec_ps[:, 0:D],
                                         mybir.ActivationFunctionType.Copy,
                                         scale=rs_sb[:, i:i + 1])

            nc.sync.dma_start(
                out=p_out[b0:b0 + BLK, :].rearrange("(i p) n -> p i n", p=128),
                in_=p_sb)
            nc.sync.dma_start(
                out=v_out[b0:b0 + BLK, :].rearrange("(i p) d -> p i d", p=128),
                in_=v_sb)

    nc.compile()
    return nc


def _prep_inputs(query, emb):
    """Host-side shard + layout prep: transpose q shard, cast to bf16."""
    q_bf = query.astype(ml_dtypes.bfloat16)
    in_maps = []
    bl = query.shape[0] // NCORES
    for c in range(NCORES):
        shard = q_bf[c * bl:(c + 1) * bl]                  # [b_loc, D]
        qt = np.ascontiguousarray(shard.T).reshape(2, 128, bl)
        in_maps.append({"qt": qt, "emb": np.ascontiguousarray(emb)})
    return in_maps


def kernel(query, emb, _trace=False, _b_loc=B_LOC, _tmpdir=None):
    assert query.shape == (B, D) and emb.shape == (N, D)
    nc = _build(_b_loc)
    in_maps = _prep_inputs(query, emb)
    if _b_loc != B_LOC:
        in_maps = [{"qt": m["qt"][:, :, :_b_loc], "emb": m["emb"]}
                   for m in in_maps]
    res = run_bass_kernel_spmd(nc, in_maps, list(range(NCORES)), trace=_trace,
                               tmpdir=_tmpdir)
    vec = np.concatenate([r["vec"] for r in res.results], axis=0)
    p = np.concatenate([r["p"] for r in res.results], axis=0)
    if _trace:
        kernel.last_exec_time_ns = res.exec_time_ns
        kernel.last_results = res
    return vec, p


# revision 16
# speedup vs baseline: 1.2861x; 1.2861x over previous
"""ConceptBank (vq_codebook) Trainium2 kernel.

reference:
    qn = l2norm(query); bankn = l2norm(emb)
    sim = qn @ bankn.T          # (B, N) cosine sims
    p = softmax(sim, -1)        # (B, N)
    vec = p @ emb               # (B, D)
    returns (vec, p)

Strategy (8 cores, data-parallel over B):
  - host pre-transposes + casts query shard to qT [2, 128, B_loc] bf16
    (layout prep only; all math on device)
  - sim tile [128b, 512n] = qT.T @ bankT  (bf16 matmul, fp32 PSUM)
  - softmax without max-subtraction (cosines are in [-1, 1]):
    ACT exp (fused per-partition 1/||q|| scale) -> e in fp32r
  - 1/||q|| via Newton rsqrt on DVE (||q||^2 ~ chi2_256 => y0=1/16
    converges quadratically); ACT never switches off the Exp/Copy LUTs
  - PE-transpose e (fp32r) -> eT; vec_u = eT.T @ [emb | 1] (fp32r);
    the ones column lands the softmax denominator s in the vec PSUM
  - p = e * (1/s) (DVE), vec = vec_u * (1/s) (DVE/ACT alternating)
"""

import numpy as np
import ml_dtypes
from contextlib import ExitStack

import concourse.bass as bass
from concourse import bacc
import concourse.mybir as mybir
import concourse.tile as tile
from concourse.bass_utils import run_bass_kernel_spmd
from concourse.masks import make_identity

F32 = mybir.dt.float32
F32R = mybir.dt.float32r
BF16 = mybir.dt.bfloat16

B, N, D = 262144, 512, 256
NCORES = 8
B_LOC = B // NCORES          # 32768 rows per core
BLK = 2048                   # rows per macro-block
NSUB = BLK // 128            # 16 subtiles of 128 rows
DE = D + 2                   # emb free dim padded with ones columns


def _build(b_loc=B_LOC):
    nblk = b_loc // BLK
    nc = bacc.Bacc(None, target_bir_lowering=False)
    qt = nc.declare_dram_parameter("qt", [2, 128, b_loc], BF16, isOutput=False)
    emb = nc.declare_dram_parameter("emb", [N, D], F32, isOutput=False)
    p_out = nc.declare_dram_parameter("p", [b_loc, N], F32, isOutput=True)
    v_out = nc.declare_dram_parameter("vec", [b_loc, D], F32, isOutput=True)

    with ExitStack() as ctx:
        tc = ctx.enter_context(tile.TileContext(nc))
        const = ctx.enter_context(tc.tile_pool(name="const", bufs=1))
        qpool = ctx.enter_context(tc.tile_pool(name="qpool", bufs=2))
        opool = ctx.enter_context(tc.tile_pool(name="opool", bufs=2))
        epool = ctx.enter_context(tc.tile_pool(name="epool", bufs=6))
        tpool = ctx.enter_context(tc.tile_pool(name="tpool", bufs=3))
        spool = ctx.enter_context(tc.tile_pool(name="spool", bufs=2))
        ps_sim = ctx.enter_context(tc.tile_pool(name="ps_sim", bufs=3, space="PSUM"))
        ps_pt = ctx.enter_context(tc.tile_pool(name="ps_pt", bufs=1, space="PSUM"))
        ps_vec = ctx.enter_context(tc.tile_pool(name="ps_vec", bufs=1, space="PSUM"))
        ps_ss = ctx.enter_context(tc.tile_pool(name="ps_ss", bufs=1, space="PSUM"))

        # ---------------- prep: bank normalize + transpose ----------------
        emb_sb = const.tile([128, 4, D], F32)          # emb rows n=128j+p
        nc.sync.dma_start(out=emb_sb,
                          in_=emb.rearrange("(j p) d -> p j d", p=128))
        # emb_ext: [emb | 1 1] rounded to fp32r (vec-matmul rhs; the ones
        # columns accumulate the softmax denominator)
        emb_ext = const.tile([128, 4, DE], F32R)
        ones_f = const.tile([128, 2], F32)
        nc.vector.memset(ones_f, 1.0)
        for j in range(4):
            nc.scalar.copy(emb_ext[:, j, 0:D], emb_sb[:, j, :])
            nc.vector.tensor_copy(emb_ext[:, j, D:DE], ones_f)

        scr = const.tile([128, D], F32)
        ssb = const.tile([128, 4], F32)
        for j in range(4):
            # sum of squares along free dim via ACT Square + fused accum
            nc.scalar.activation(scr, emb_sb[:, j, :],
                                 mybir.ActivationFunctionType.Square,
                                 accum_out=ssb[:, j:j + 1])
        nbn = const.tile([128, 4], F32)
        nc.scalar.sqrt(nbn, ssb)
        rbn = const.tile([128, 4], F32)
        nc.vector.reciprocal(rbn, nbn)

        bankn = const.tile([128, 4, D], BF16)          # normalized bank, bf16
        for j in range(4):
            nc.vector.tensor_scalar_mul(bankn[:, j, :], emb_sb[:, j, :],
                                        rbn[:, j:j + 1])

        ident_h = const.tile([128, 128], BF16)
        make_identity(nc, ident_h)
        ident_f = const.tile([128, 128], F32)
        make_identity(nc, ident_f)
        ident_r = const.tile([128, 128], F32R)
        nc.vector.tensor_copy(ident_r, ident_f)
        ones_h = const.tile([128, 1], BF16)
        nc.vector.memset(ones_h, 1.0)

        # bankT [128d, k, 512n] bf16: transpose of bankn
        bankT = const.tile([128, 2, N], BF16)
        for j in range(4):
            for k in range(2):
                tp = ps_pt.tile([128, 128], BF16, tag="pt")
                nc.tensor.transpose(tp, bankn[:, j, 128 * k:128 * (k + 1)],
                                    ident_h)
                nc.vector.tensor_copy(bankT[:, k, 128 * j:128 * (j + 1)], tp)

        # ---------------- main loop ----------------
        for blk in range(nblk):
            b0 = blk * BLK
            qt_sb = qpool.tile([128, 2, BLK], BF16)
            nc.sync.dma_start(out=qt_sb, in_=qt[:, :, b0:b0 + BLK]
                              .rearrange("k p b -> p k b"))
            # squares for ||q||^2 (bf16: fine, no cancellation in the sum)
            qsq = qpool.tile([128, 2, BLK], BF16)
            nc.vector.tensor_mul(qsq, qt_sb, qt_sb)

            # ss[:, i] = sum_d qT[d, b]^2 for subtile i (PE ones-matmul)
            ss_ps = ps_ss.tile([128, NSUB], F32)
            for i in range(NSUB):
                for k in range(2):
                    nc.tensor.matmul(ss_ps[:, i:i + 1],
                                     qsq[:, k, 128 * i:128 * (i + 1)],
                                     ones_h, start=(k == 0), stop=(k == 1))
            # rq = 1/sqrt(ss) via Newton on DVE (ACT stays on Exp/Copy LUTs)
            ss_sb = spool.tile([128, NSUB], F32)
            nc.vector.tensor_copy(ss_sb, ss_ps)
            rq = spool.tile([128, NSUB], F32)
            nc.vector.memset(rq, 0.0625)               # 1/sqrt(256)
            t0 = spool.tile([128, NSUB], F32)
            for _ in range(3):
                nc.vector.tensor_mul(t0, rq, rq)       # y^2
                nc.vector.tensor_mul(t0, t0, ss_sb)    # a*y^2
                nc.vector.tensor_scalar(t0, t0, -0.5, 1.5,
                                        op0=mybir.AluOpType.mult,
                                        op1=mybir.AluOpType.add)
                nc.vector.tensor_mul(rq, rq, t0)       # y *= 1.5 - a*y^2/2

            p_sb = opool.tile([128, NSUB, N], F32)
            v_sb = opool.tile([128, NSUB, D], F32)
            rs_sb = spool.tile([128, NSUB], F32)

            for t in range(NSUB // 2):
                pt_ps = ps_pt.tile([128, 2 * N], F32R, tag="pt")
                vec_ps = ps_vec.tile([128, 2, 512], F32)  # bank-aligned halves
                e_pair = []
                for h in range(2):
                    i = 2 * t + h
                    # sim = qT.T @ bankT  (accumulate over 2 d-chunks)
                    sim_ps = ps_sim.tile([128, N], F32)
                    for k in range(2):
                        nc.tensor.matmul(sim_ps,
                                         qt_sb[:, k, 128 * i:128 * (i + 1)],
                                         bankT[:, k, :],
                                         start=(k == 0), stop=(k == 1))
                    # e = exp(sim/||q||), rounded to fp32r
                    e_sb = epool.tile([128, N], F32R)
                    nc.scalar.activation(e_sb, sim_ps,
                                         mybir.ActivationFunctionType.Exp,
                                         scale=rq[:, i:i + 1])
                    e_pair.append(e_sb)
                    # eT via PE transpose (fp32r)
                    for j in range(4):
                        nc.tensor.transpose(
                            pt_ps[:, h * N + 128 * j:h * N + 128 * (j + 1)],
                            e_sb[:, 128 * j:128 * (j + 1)],
                            ident_r)
                # one batched PSUM->SBUF copy for both subtiles
                pt_sb = tpool.tile([128, 2 * N], F32R)
                if t % 2 == 0:
                    nc.scalar.copy(pt_sb, pt_ps)
                else:
                    nc.vector.tensor_copy(pt_sb, pt_ps)
                for h in range(2):
                    i = 2 * t + h
                    # vec_u = eT.T @ [emb|1] (fp32r, accumulate 4 n-chunks);
                    # column D collects s = sum_n e
                    for j in range(4):
                        nc.tensor.matmul(
                            vec_ps[:, h, 0:DE],
                            pt_sb[:, h * N + 128 * j:h * N + 128 * (j + 1)],
                            emb_ext[:, j, :],
                            start=(j == 0), stop=(j == 3))
                # rs = 1/s for the pair
                nc.vector.reciprocal(
                    rs_sb[:, 2 * t:2 * t + 2],
                    vec_ps[:, :, D:D + 1].rearrange("p a o -> p (a o)"))
                for h in range(2):
                    i = 2 * t + h
                    # p = e * (1/s)
                    nc.vector.tensor_scalar_mul(p_sb[:, i, :], e_pair[h],
                                                rs_sb[:, i:i + 1])
                    # vec = vec_u * (1/s)  (alternate DVE / ACT-Copy)
                    if h == 0:
                        nc.scalar.activation(v_sb[:, i, :], vec_ps[:, h, 0:D],
                                             mybir.ActivationFunctionType.Copy,
                                             scale=rs_sb[:, i:i + 1])
                    else:
                        nc.vector.tensor_scalar_mul(v_sb[:, i, :],
                                                    vec_ps[:, h, 0:D],
                                                    rs_sb[:, i:i + 1])

            nc.sync.dma_start(
                out=p_out[b0:b0 + BLK, :].rearrange("(i p) n -> p i n", p=128),
                in_=p_sb)
            nc.sync.dma_start(
                out=v_out[b0:b0 + BLK, :].rearrange("(i p) d -> p i d", p=128),
                in_=v_sb)

    nc.compile()
    return nc


def _prep_inputs(query, emb):
    """Host-side shard + layout prep: transpose q shard, cast to bf16."""
    q_bf = query.astype(ml_dtypes.bfloat16)
    in_maps = []
    bl = query.shape[0] // NCORES
    for c in range(NCORES):
        shard = q_bf[c * bl:(c + 1) * bl]                  # [b_loc, D]
        qt = np.ascontiguousarray(shard.T).reshape(2, 128, bl)
        in_maps.append({"qt": qt, "emb": np.ascontiguousarray(emb)})
    return in_maps


def kernel(query, emb, _trace=False, _b_loc=B_LOC, _tmpdir=None):
    assert query.shape == (B, D) and emb.shape == (N, D)
    nc = _build(_b_loc)
    in_maps = _prep_inputs(query, emb)
    if _b_loc != B_LOC:
        in_maps = [{"qt": m["qt"][:, :, :_b_loc], "emb": m["emb"]}
                   for m in in_maps]
    res = run_bass_kernel_spmd(nc, in_maps, list(range(NCORES)), trace=_trace,
                               tmpdir=_tmpdir)
    vec = np.concatenate([r["vec"] for r in res.results], axis=0)
    p = np.concatenate([r["p"] for r in res.results], axis=0)
    if _trace:
        kernel.last_exec_time_ns = res.exec_time_ns
        kernel.last_results = res
    return vec, p
